# revision 33
# baseline (speedup 1.0000x reference)
"""Trainium2 Bass kernel for nn_Patchifier (grid-cell argmax + clamped top-k).

Per (b, n) map of shape [480, 640] (128 maps total, 16 per core):
  - 10x10 grid of 48x64 cells; per-cell argmax (first occurrence, row-major)
  - clamp coords to [1, 478] x [1, 638]; re-read score at the clamped coord
  - top-80 of the 100 candidate scores, sorted descending, jax tie order
Returns (x_coords i32 [4,32,80], y_coords i32, top_scores f32).

v3 dataflow per core (over the v2 relay design):
  1. per map: HBM -> SBUF raw [120, 2560]; the 64-col rowmax scan alternates
     engines per map -- DVE tensor_reduce for some maps, a Pool-engine
     tensor_tensor max fold chain (64->32->16->8->4->2->1) for the rest --
     so neither vector engine exceeds the DMA roofline.
  2. PE transpose of the [120, 40] rowmax block, then ONE SBUF->SBUF DMA
     (48B elems) scatters it straight into the [100 cells, 48] layout of
     cellsX (no DRAM relay roundtrip).
  3. per group of 4 maps: cellmax + first-attaining-row (value-iota min
     trick) -> rmin; ONE batched indirect gather (SWDGE) fetches all four
     maps' argmax-row 64-col chunks.
  4. batched column argmax, fused Relu clamp chains, one-hot score
     extraction with border-cell blend via pm2 matmul (as v2).
  5. rank sort per group right after its post stage (PE comparison matrix
     + eps tie-break, DVE win-count, one-hot(rank) matmul gather).
  6. tail: PE-transpose the sorted [80, (m,e)] tile to [(m,e), 80] and
     write x/y/s with three contiguous 320B-elem DMAs.
"""

import sys

if "/opt/trn_rl_repo" not in sys.path:
    sys.path.insert(0, "/opt/trn_rl_repo")

import numpy as np

import concourse.bacc as bacc
import concourse.bass as bass
import concourse.mybir as mybir
from concourse.bass import IndirectOffsetOnAxis, broadcast_tensor_aps
from concourse.instruction_name_ordered_set import InstructionNameOrderedSet
from concourse.tile import TileContext
from concourse.bass_utils import run_bass_kernel_spmd

F32 = mybir.dt.float32
I32 = mybir.dt.int32

N_CORES = 8
MAPS = 16
H, W = 480, 640
G10 = 10
CH, CW = 48, 64
NCAND = 100
TOPN = 80
GROUPS = [(0, 4), (4, 4), (8, 4), (12, 4)]
# which engine scans each map: 'd' = DVE tensor_reduce, 'p' = Pool fold chain
SCAN = "dddddddddddddddd"
EPS_TIE = 4e-6
AX = mybir.AluOpType
AF = mybir.ActivationFunctionType

# cpack column blocks: name -> (col0, width)
_CPACK = {
    "viota48": (0, CH),       # 4*(f%12) + f//12 - 1000
    "iota64m": (48, CW),      # c - 1000
    "iota80": (112, TOPN),    # k
    "bxrow": (192, 1),        # 999 + 48g
    "bycol": (193, 1),        # 999 + 64gc
    "b477": (194, 1),
    "b637": (195, 1),
    "negcgc64": (196, 1),     # -64gc
    "ones100": (197, 1),
    "selc": (198, 1),         # rmin value that means "row got clamped"
    "iota64c": (199, CW),     # c
    "bigmoff": (263, MAPS),   # 10000 + 480g + 4800m + gc
    "iota80r": (263 + MAPS, TOPN),  # 99 - k (rank = 99 - wins)
}
_CPACK_W = 263 + MAPS + TOPN
NX = 120


def _consts():
    p = np.arange(NCAND)
    g = p // G10
    gc = p % G10
    f = np.arange(CH)
    cp = np.zeros((128, _CPACK_W), dtype=np.float32)
    cp[:, 0:48] = (4 * (f % 12) + f // 12 - 1000)[None, :]
    cp[:, 48:112] = (np.arange(CW) - 1000)[None, :]
    cp[:, 112:192] = np.arange(TOPN, dtype=np.float32)[None, :]
    cp[:NCAND, 192] = 999 + CH * g
    cp[:NCAND, 193] = 999 + CW * gc
    cp[:, 194] = 477.0
    cp[:, 195] = 637.0
    cp[:NCAND, 196] = -CW * gc
    cp[:, 197] = 1.0
    # sel: row-clamped iff rmin == -1000 (g=0) or rmin == -953 (g=9)
    cp[:NCAND, 198] = np.where(g == 0, -1000.0,
                               np.where(g == 9, -953.0, 5.0))
    cp[:, 199:263] = np.arange(CW, dtype=np.float32)[None, :]
    cp[:NCAND, 263:263 + MAPS] = (
        10000 + 480 * g[:, None] + 4800 * np.arange(MAPS)[None, :]
        + gc[:, None])
    cp[:, 263 + MAPS:263 + MAPS + TOPN] = (
        99 - np.arange(TOPN, dtype=np.float32))[None, :]
    c = {"cpack": cp}
    c["ident"] = np.eye(128, dtype=np.float32)
    c["ltepsT"] = (EPS_TIE * (p[:, None] < p[None, :])).astype(np.float32)
    e = np.zeros((4, 4 * NCAND), dtype=np.float32)
    for k in range(4):
        e[k, k * NCAND:(k + 1) * NCAND] = 1.0
    c["esel"] = e
    # pm2: [0:100, 0:120] = Pdup (dup border cells' value onto rows 100-119,
    # identity on 0-99); [0:120, 120:220] = Psel2 (select the border-row
    # score for border cells, zero for interior)
    pm2 = np.zeros((128, 220), dtype=np.float32)
    pm2[np.arange(NCAND), np.arange(NCAND)] = 1.0
    for i in range(10):
        pm2[i, 100 + i] = 1.0            # cell i -> partition 100+i
        pm2[90 + i, 110 + i] = 1.0       # cell 90+i -> partition 110+i
        pm2[100 + i, 120 + i] = 1.0      # Psel2: border row for cells 0-9
        pm2[110 + i, 120 + 90 + i] = 1.0  # ... and cells 90-99
    c["pm2"] = pm2
    # constant gather rows for partitions 100-119: row 1 (p=100+i, col
    # block gc=i) and row 478 (p=110+i); chunk idx = row*10 + gc + 4800m
    ib = np.zeros((20, MAPS), dtype=np.int32)
    for i in range(10):
        ib[i, :] = 10 + i + 4800 * np.arange(MAPS)
        ib[10 + i, :] = 4780 + i + 4800 * np.arange(MAPS)
    c["idxbrd"] = ib
    return c


_CONST_SHAPES = {
    "cpack": [128, _CPACK_W], "ident": [128, 128],
    "ltepsT": [NCAND, NCAND], "esel": [4, 4 * NCAND], "pm2": [128, 220],
}


def _bc(a, b):
    return broadcast_tensor_aps(a, b)


def build_nc():
    nc = bacc.Bacc()

    score = nc.dram_tensor("score", [MAPS, H, W], F32, kind="ExternalInput")
    cdecl = {
        name: nc.dram_tensor(name, shape, F32, kind="ExternalInput")
        for name, shape in _CONST_SHAPES.items()
    }
    x_out = nc.dram_tensor("x_out", [MAPS, TOPN], I32, kind="ExternalOutput")
    y_out = nc.dram_tensor("y_out", [MAPS, TOPN], I32, kind="ExternalOutput")
    s_out = nc.dram_tensor("s_out", [MAPS, TOPN], F32, kind="ExternalOutput")

    # chunk view (64 contiguous floats) of the input
    score_chunks = score[:, :, :].rearrange("m r (gc c) -> (m r gc) c", c=CW)
    idxbrd = nc.dram_tensor("idxbrd", [20, MAPS], I32, kind="ExternalInput")

    # DRAM relay: addr = 480g + 48gc + 12r + s -> [cell, f=12r+s] read-back
    relay = nc.dram_tensor("relay", [MAPS, NCAND * CH], F32)

    with TileContext(nc) as tc:
        with (
            tc.tile_pool(name="raw", bufs=10) as rawp,
            tc.tile_pool(name="rm", bufs=4) as rmp,
            tc.tile_pool(name="fold", bufs=2) as foldp,
            tc.tile_pool(name="rt", bufs=3) as rtp,
            tc.tile_pool(name="jk", bufs=3) as jkp,
            tc.tile_pool(name="small", bufs=1) as sp,
            tc.tile_pool(name="psA", bufs=3, space="PSUM") as ppA,
            tc.tile_pool(name="psB", bufs=1, space="PSUM") as ppB,
        ):
            ctile = {}
            for name, d in cdecl.items():
                t = sp.tile(list(d.shape), F32, tag=name)
                nc.scalar.dma_start(out=t[:, :], in_=d[:, :])
                ctile[name] = t
            ident = ctile["ident"]

            def C(name, f=None, pn=NCAND):
                c0, w = _CPACK[name]
                f0, f1 = (0, w) if f is None else f
                return ctile["cpack"][0:pn, c0 + f0:c0 + f1]

            # long-lived state
            cellsX = sp.tile([NCAND, MAPS * CH], F32, tag="cellsX")
            chunkCat = sp.tile([NX, MAPS * CW], F32, tag="chunkCat")
            cmx = sp.tile([NCAND, MAPS], F32, tag="cmx")
            rminf = sp.tile([NCAND, MAPS], F32, tag="rminf")
            xt1 = sp.tile([NCAND, MAPS], F32, tag="xt1")
            xt2 = sp.tile([NCAND, MAPS], F32, tag="xt2")
            idxf = sp.tile([NCAND, MAPS], F32, tag="idxf")
            idxi = sp.tile([NX, MAPS], I32, tag="idxi")
            nc.scalar.dma_start(out=idxi[NCAND:NX, :], in_=idxbrd[:, :])
            cm64 = sp.tile([NCAND, MAPS], F32, tag="cm64")
            cminf = sp.tile([NCAND, MAPS], F32, tag="cminf")
            yt1 = sp.tile([NCAND, MAPS], F32, tag="yt1")
            yt2 = sp.tile([NCAND, MAPS], F32, tag="yt2")
            cclv = sp.tile([NCAND, MAPS], F32, tag="cclv")
            cclx = sp.tile([NX, MAPS], F32, tag="cclx")
            scoreX = sp.tile([NX, MAPS], F32, tag="scoreX")
            selv = sp.tile([NCAND, MAPS], F32, tag="selv")
            bd = sp.tile([NCAND, MAPS], F32, tag="bd")
            be = sp.tile([NCAND, MAPS], F32, tag="be")
            S3v = sp.tile([NCAND, 3 * MAPS], F32, tag="S3v")
            sc16 = sp.tile([NCAND, MAPS], F32, tag="sc16")
            rows4s = sp.tile([4, NCAND], F32, tag="rows4s")
            rankf = sp.tile([NCAND, MAPS], F32, tag="rankf")
            srt = sp.tile([TOPN, 3 * MAPS], F32, tag="srt")
            outS = sp.tile([3 * MAPS, TOPN], F32, tag="outS")
            outI = sp.tile([3 * MAPS, TOPN], I32, tag="outI")

            psmall = ppB.tile([NX, 132], F32, tag="psmall")
            cclP = psmall[:, 0:MAPS]
            rows4P = psmall[0:4, 32:132]
            sortP = ppB.tile([TOPN, 3 * MAPS], F32, tag="sortP")

            c3 = lambda t, f: t.rearrange("p (m f) -> p m f", f=f)
            c1 = lambda t: t.rearrange("p (m o) -> p m o", o=1)

            from contextlib import contextmanager

            @contextmanager
            def at_prio(p):
                old = tc.cur_priority
                tc.cur_priority = p
                try:
                    yield
                finally:
                    tc.cur_priority = old

            BAND = 100

            def load_map(m):
                r = rawp.tile([120, 2560], F32, tag="raw")
                rv = r[:, :].rearrange("p (r w) -> p r w", r=4)
                for h in range(2):
                    nc.sync.dma_start(
                        out=rv[:, :, 320 * h:320 * (h + 1)],
                        in_=score[m:m + 1, :, 320 * h:320 * (h + 1)].rearrange(
                            "mm (p rr) c -> p (mm rr) c", rr=4),
                    )
                return r

            def _after(inst, anchor):
                if anchor is None:
                    return inst
                s = InstructionNameOrderedSet()
                s.add(anchor.ins.name)
                inst.ins.add_nosync_dependencies_from(s)
                return inst

            def scan_dve(raw, rm, h):
                g5 = slice(5 * h, 5 * (h + 1))
                return nc.vector.reduce_max(
                    out=rm[:, g5, :].rearrange("p gc r -> p r gc"),
                    in_=raw[:, :].rearrange("p (r gc c) -> p r gc c",
                                            r=4, gc=G10)[:, :, g5, :],
                    axis=mybir.AxisListType.X,
                )

            def scan_pool(raw, rm, fold):
                # full-map fold chain; all APs stay <= 3 dims (q = (r gc))
                rv3 = raw[:, :].rearrange("p (q c) -> p q c", c=CW)
                f1 = fold[:, 0:1280].rearrange("p (q c) -> p q c", c=32)
                f2 = fold[:, 1280:1920].rearrange("p (q c) -> p q c", c=16)
                f3 = fold[:, 1920:2240].rearrange("p (q c) -> p q c", c=8)
                f4 = fold[:, 2240:2400].rearrange("p (q c) -> p q c", c=4)
                f5 = fold[:, 2400:2480].rearrange("p (t q) -> p q t", t=2)
                # (t outer in memory so the final fold reads two flat planes)
                nc.gpsimd.tensor_tensor(
                    out=f1, in0=rv3[:, :, 0:32], in1=rv3[:, :, 32:64],
                    op=AX.max)
                nc.gpsimd.tensor_tensor(
                    out=f2, in0=f1[:, :, 0:16], in1=f1[:, :, 16:32],
                    op=AX.max)
                nc.gpsimd.tensor_tensor(
                    out=f3, in0=f2[:, :, 0:8], in1=f2[:, :, 8:16],
                    op=AX.max)
                nc.gpsimd.tensor_tensor(
                    out=f4, in0=f3[:, :, 0:4], in1=f3[:, :, 4:8],
                    op=AX.max)
                nc.gpsimd.tensor_tensor(
                    out=f5, in0=f4[:, :, 0:2], in1=f4[:, :, 2:4],
                    op=AX.max)
                return nc.gpsimd.tensor_tensor(
                    out=rm[:, :, :].rearrange("p gc r -> p r gc"),
                    in0=f5[:, :, 0:1].rearrange(
                        "p (r gc) t -> p r (gc t)", r=4),
                    in1=f5[:, :, 1:2].rearrange(
                        "p (r gc) t -> p r (gc t)", r=4),
                    op=AX.max)

            anchors = {}

            def rowmax_stage(m, raw):
                rm = rmp.tile([120, G10, 4], F32, tag="rm")
                scan_dve(raw, rm, 0)
                anchors[m] = scan_dve(raw, rm, 1)
                ps = ppA.tile([40, 120], F32, tag="ps_rt")
                nc.tensor.transpose(
                    out=ps[:, :],
                    in_=rm[:, :, :].rearrange("p gc r -> p (gc r)"),
                    identity=ident[:120, :120])
                rt = rtp.tile([40, 120], F32, tag="rt")
                nc.scalar.copy(out=rt[:, :], in_=ps[:, :])
                # relay_out pairs with the copy on the Act queue -- fires
                # immediately after it, never blocking the SP load stream
                nc.scalar.dma_start(
                    out=relay[m:m + 1, :].rearrange(
                        "mm (g2 q s) -> (mm q) g2 s", g2=G10, q=40, s=12),
                    in_=rt[:, :].rearrange("q (g2 s) -> q g2 s", g2=G10),
                )
                return rt

            def relay_out(m, rt):
                nc.sync.dma_start(
                    out=relay[m:m + 1, :].rearrange(
                        "mm (g2 q s) -> (mm q) g2 s", g2=G10, q=40, s=12),
                    in_=rt[:, :].rearrange("q (g2 s) -> q g2 s", g2=G10),
                )

            def relay_in(m0, k):
                sl48 = slice(m0 * CH, (m0 + k) * CH)
                nc.scalar.dma_start(
                    out=cellsX[:, sl48].rearrange("p (m f) -> p m f", f=CH),
                    in_=relay[m0:m0 + k, :].rearrange(
                        "mm (cell f) -> cell mm f", cell=NCAND),
                )

            def group_pre(m0, k, anchor=None, pool_tt=False):
                """cellmax -> rmin -> gather indices (DVE part only)."""
                sl = slice(m0, m0 + k)
                sl48 = slice(m0 * CH, (m0 + k) * CH)

                _after(nc.vector.reduce_max(
                    out=cmx[:, sl], in_=c3(cellsX[:, sl48], CH),
                    axis=mybir.AxisListType.X), anchor)
                jR1 = jkp.tile([NCAND, 4 * CH], F32, tag="jR1")
                a, b = _bc(c3(cellsX[:, sl48], CH), c1(cmx[:, sl]))
                nc.vector.tensor_tensor(
                    out=c3(jR1[:, 0:k * CH], CH), in0=a, in1=b, op=AX.is_ge)
                jR2 = jkp.tile([NCAND, 4 * CH], F32, tag="jR2")
                a, b = _bc(c3(jR1[:, 0:k * CH], CH),
                           C("viota48").rearrange("p (o f) -> p o f", o=1))
                eng2 = nc.gpsimd if pool_tt else nc.vector
                eng2.tensor_tensor(
                    out=c3(jR2[:, 0:k * CH], CH), in0=a, in1=b, op=AX.mult)
                nc.vector.tensor_reduce(
                    out=rminf[:, sl], in_=c3(jR2[:, 0:k * CH], CH),
                    axis=mybir.AxisListType.X, op=AX.min)

                # chunk idx = rmin*10 + (10000 + 480g + 4800m + gc);
                # partitions 100-119 use constant rows 1 / 478 (preloaded).
                # Per-map Act ops (scale/bias fusion) keep this off the DVE.
                for m in range(m0, m0 + k):
                    nc.scalar.activation(
                        idxf[:, m:m + 1], rminf[:, m:m + 1], AF.Relu,
                        bias=C("bigmoff", f=(m, m + 1)), scale=10.0)
                nc.vector.tensor_copy(idxi[0:NCAND, sl], idxf[:, sl])

            def group_gather(m0, k, anchor=None):
                for m in range(m0, m0 + k):
                    _after(nc.gpsimd.indirect_dma_start(
                        out=chunkCat[:, m * CW:(m + 1) * CW],
                        out_offset=None, in_=score_chunks,
                        in_offset=IndirectOffsetOnAxis(
                            ap=idxi[:, m:m + 1], axis=0),
                    ), anchor)

            def group_post(m0, k, anchor=None, pool_tt=False):
                """column argmax -> clamp -> score blend."""
                sl = slice(m0, m0 + k)
                sl64 = slice(m0 * CW, (m0 + k) * CW)

                # row clamp chain (x output; off the gather path)
                nc.scalar.activation(xt1[:, sl], rminf[:, sl], AF.Relu,
                                     bias=C("bxrow"))
                nc.scalar.activation(xt2[:, sl], xt1[:, sl], AF.Relu,
                                     bias=C("b477"), scale=-1.0)
                s3 = S3v[:, 3 * m0:3 * (m0 + k)].rearrange(
                    "p (m e) -> p m e", e=3)
                nc.scalar.activation(s3[:, :, 1:2], c1(xt2[:, sl]), AF.Copy,
                                     bias=float(H - 2), scale=-1.0)

                _after(nc.vector.reduce_max(
                    out=cm64[:, sl], in_=c3(chunkCat[0:NCAND, sl64], CW),
                    axis=mybir.AxisListType.X), anchor)
                j641 = jkp.tile([NCAND, 4 * CW], F32, tag="j641")
                a, b = _bc(c3(chunkCat[0:NCAND, sl64], CW), c1(cm64[:, sl]))
                nc.vector.tensor_tensor(
                    out=c3(j641[:, 0:k * CW], CW), in0=a, in1=b, op=AX.is_ge)
                j642 = jkp.tile([NCAND, 4 * CW], F32, tag="j642")
                a, b = _bc(c3(j641[:, 0:k * CW], CW),
                           C("iota64m").rearrange("p (o f) -> p o f", o=1))
                eng2 = nc.gpsimd if pool_tt else nc.vector
                eng2.tensor_tensor(
                    out=c3(j642[:, 0:k * CW], CW), in0=a, in1=b, op=AX.mult)
                nc.vector.tensor_reduce(
                    out=cminf[:, sl], in_=c3(j642[:, 0:k * CW], CW),
                    axis=mybir.AxisListType.X, op=AX.min)

                # col clamp chain: ysl = clip(cmin+1000+64gc, 1, 638);
                # ccl = ysl - 64gc (column within the 64-chunk)
                nc.scalar.activation(yt1[:, sl], cminf[:, sl], AF.Relu,
                                     bias=C("bycol"))
                nc.scalar.activation(yt2[:, sl], yt1[:, sl], AF.Relu,
                                     bias=C("b637"), scale=-1.0)
                s3y = S3v[:, 3 * m0:3 * (m0 + k)].rearrange(
                    "p (m e) -> p m e", e=3)[:, :, 2:3]
                nc.scalar.activation(s3y, c1(yt2[:, sl]), AF.Copy,
                                     bias=float(W - 2), scale=-1.0)
                nc.scalar.activation(c1(cclv[:, sl]), s3y, AF.Relu,
                                     bias=C("negcgc64"))

                # duplicate border cells' ccl onto partitions 100-119
                nc.tensor.matmul(
                    out=cclP[:, sl], lhsT=ctile["pm2"][0:NCAND, 0:NX],
                    rhs=cclv[:, sl], start=True, stop=True)
                nc.scalar.copy(out=cclx[:, sl], in_=cclP[:, sl])

                # score extraction on all 120 rows: one-hot(ccl) . chunk
                jS1 = jkp.tile([NX, 4 * CW], F32, tag="jS1")
                a, b = _bc(C("iota64c", pn=NX).rearrange(
                    "p (o f) -> p o f", o=1), c1(cclx[:, sl]))
                nc.vector.tensor_tensor(
                    out=c3(jS1[:, 0:k * CW], CW), in0=a, in1=b,
                    op=AX.is_equal)
                jS2 = jkp.tile([NX, 4 * CW], F32, tag="jS2")
                eng2.tensor_tensor(
                    out=jS2[:, 0:k * CW], in0=jS1[:, 0:k * CW],
                    in1=chunkCat[:, sl64], op=AX.mult)
                nc.vector.tensor_reduce(
                    out=scoreX[:, sl], in_=c3(jS2[:, 0:k * CW], CW),
                    axis=mybir.AxisListType.X, op=AX.add)
                # border-row score per cell (zero for interior)
                sbP = psmall[:, 16:32]
                nc.tensor.matmul(
                    out=sbP[0:NCAND, sl], lhsT=ctile["pm2"][0:NX, 120:220],
                    rhs=scoreX[:, sl], start=True, stop=True)
                # blend: row-clamped cells take the border-row value
                a, b = _bc(c1(rminf[:, sl]), C("selc").rearrange(
                    "p (o f) -> p o f", o=1))
                nc.vector.tensor_tensor(
                    out=c1(selv[:, sl]), in0=a, in1=b, op=AX.is_equal)
                nc.vector.tensor_tensor(
                    out=bd[:, sl], in0=sbP[0:NCAND, sl],
                    in1=scoreX[0:NCAND, sl], op=AX.subtract)
                nc.vector.tensor_tensor(
                    out=be[:, sl], in0=bd[:, sl], in1=selv[:, sl],
                    op=AX.mult)
                nc.vector.tensor_tensor(
                    out=sc16[:, sl], in0=be[:, sl],
                    in1=scoreX[0:NCAND, sl], op=AX.add)
                s3s = S3v[:, 3 * m0:3 * (m0 + k)].rearrange(
                    "p (m e) -> p m e", e=3)[:, :, 0:1]
                nc.vector.tensor_copy(s3s, c1(sc16[:, sl]))

            def rank_stage(m0, k):
                sl = slice(m0, m0 + k)
                nc.tensor.matmul(
                    out=rows4P[0:k, :], lhsT=sc16[:, sl],
                    rhs=ident[:NCAND, :NCAND], start=True, stop=True)
                nc.scalar.copy(out=rows4s[0:k, :], in_=rows4P[0:k, :])
                aallP = ppB.tile([NCAND, 4 * NCAND], F32, tag="aallP",
                                 bufs=2)
                for j in range(k):
                    asl = slice(j * NCAND, (j + 1) * NCAND)
                    nc.tensor.matmul(
                        out=aallP[:, asl],
                        lhsT=ctile["esel"][0:k, asl],
                        rhs=rows4s[0:k, :], start=True, stop=False)
                    nc.tensor.matmul(
                        out=aallP[:, asl], lhsT=ctile["ltepsT"][:, :],
                        rhs=ident[:NCAND, :NCAND], start=False, stop=True)
                jG = jkp.tile([NCAND, 4 * NCAND], F32, tag="jG")
                a, b = _bc(c3(aallP[:, 0:k * NCAND], NCAND), c1(sc16[:, sl]))
                nc.vector.tensor_tensor(
                    out=c3(jG[:, 0:k * NCAND], NCAND), in0=a, in1=b,
                    op=AX.is_gt)
                nc.vector.tensor_reduce(
                    out=rankf[:, sl], in_=c3(jG[:, 0:k * NCAND], NCAND),
                    axis=mybir.AxisListType.X, op=AX.add)
                jO = jkp.tile([NCAND, 4 * TOPN], F32, tag="jO")
                a, b = _bc(C("iota80").rearrange("p (o f) -> p o f", o=1),
                           c1(rankf[:, sl]))
                nc.vector.tensor_tensor(
                    out=c3(jO[:, 0:k * TOPN], TOPN), in0=a, in1=b,
                    op=AX.is_equal)
                for j in range(k):
                    m = m0 + j
                    nc.tensor.matmul(
                        out=sortP[:, 3 * m:3 * m + 3],
                        lhsT=jO[:, j * TOPN:(j + 1) * TOPN],
                        rhs=S3v[:, 3 * m:3 * m + 3],
                        start=True, stop=True)

            # ---- main schedule ----
            # Loads stream on the SP queue with nothing else on it; per-map
            # relays ride the Act queue; gathers (SWDGE) + half the scans on
            # Pool. The gather for group g is emitted one step after its
            # index computation so it never parks long at the in-order Pool
            # SEQ head (which would stall the fold scans behind it).
            # Hand-scheduled pipeline. Bands place each stage into a
            # computed idle slot of its engine; queues are in-order so a
            # stage waits only where its engine is naturally idle.
            # SP runs loads + early relays (SEQ runs ahead of transfers,
            # so relay waits at SP head have ~8us of slack); tail maps'
            # relay_outs pair with their copies on Act.
            PRE_BAND = {0: 1650, 1: 2050, 2: 2450, 3: 2742}
            RIN_BAND = {0: 2705, 1: 2715, 2: 2725, 3: 2741}
            GATHER_BAND = {0: 1700, 1: 2100, 2: 2500, 3: 2750}
            POST_BAND = {0: 1950, 1: 2350, 2: 2550, 3: 2760}
            RANK_BAND = {0: 1980, 1: 2380, 2: 2570, 3: 2770}

            rts = {}
            last_to_group = {GROUPS[gi][0] + GROUPS[gi][1] - 1: gi
                             for gi in range(len(GROUPS))}

            PRE_ANCHOR = {0: 6, 1: 10, 2: 14, 3: 15}
            POST_ANCHOR = {0: 9, 1: 13, 2: 15, 3: None}
            GATHER_ANCHOR = {0: None, 1: None, 2: None, 3: None}

            def emit_group(gi):
                m0, k = GROUPS[gi]
                pa = PRE_ANCHOR.get(gi)
                oa = POST_ANCHOR.get(gi)
                ga = GATHER_ANCHOR.get(gi)
                with at_prio(RIN_BAND[gi]):
                    relay_in(m0, k)
                with at_prio(PRE_BAND[gi]):
                    group_pre(m0, k, anchors.get(pa))
                with at_prio(GATHER_BAND[gi]):
                    group_gather(m0, k, anchors.get(ga))
                with at_prio(POST_BAND[gi]):
                    group_post(m0, k, anchors.get(oa))
                with at_prio(RANK_BAND[gi]):
                    rank_stage(m0, k)

            for m in range(MAPS):
                with at_prio(1000 + BAND * m):
                    raw = load_map(m)
                    rts[m] = rowmax_stage(m, raw)
                gi = last_to_group.get(m)
                if gi is not None:
                    # relay_in directly behind this group's last relay_out
                    # on the Act queue: fires as soon as the relay lands
                    with at_prio(1000 + BAND * m + 70):
                        relay_in(*GROUPS[gi])
            for gi in range(len(GROUPS)):
                emit_group(gi)

            # ---- tail: copy, transpose, then 3 contiguous DMAs ----
            nc.scalar.copy(out=srt[:, :], in_=sortP[:, :])
            outP = ppA.tile([3 * MAPS, TOPN], F32, tag="outP", bufs=1)
            nc.tensor.transpose(
                out=outP[:, :], in_=srt[:, :], identity=ident[:TOPN, :TOPN])
            nc.scalar.copy(out=outS[:, :], in_=outP[:, :])
            nc.vector.tensor_copy(outI[:, :], outS[:, :])
            o3f = outS[:, :].rearrange("(m e) k -> m e k", e=3)
            o3i = outI[:, :].rearrange("(m e) k -> m e k", e=3)
            nc.sync.dma_start(
                out=s_out[:, :].rearrange("m (o k) -> m o k", o=1),
                in_=o3f[:, 0:1, :])
            nc.scalar.dma_start(
                out=x_out[:, :].rearrange("m (o k) -> m o k", o=1),
                in_=o3i[:, 1:2, :])
            nc.scalar.dma_start(
                out=y_out[:, :].rearrange("m (o k) -> m o k", o=1),
                in_=o3i[:, 2:3, :])

    nc.compile()
    return nc


_NC = None


def _get_nc():
    global _NC
    if _NC is None:
        _NC = build_nc()
    return _NC


def kernel(score_maps: np.ndarray, top_n=80, _trace=False):
    score_maps = np.ascontiguousarray(np.asarray(score_maps), dtype=np.float32)
    assert score_maps.shape == (4, 32, H, W), score_maps.shape
    assert int(top_n) == TOPN

    nc = _get_nc()
    consts = _consts()
    flat = score_maps.reshape(4 * 32, H, W)
    in_maps = []
    for c in range(N_CORES):
        m = {"score": np.ascontiguousarray(flat[c * MAPS:(c + 1) * MAPS])}
        m.update(consts)
        in_maps.append(m)

    res = run_bass_kernel_spmd(nc, in_maps, list(range(N_CORES)), trace=_trace)
    x = np.concatenate([res.results[c]["x_out"] for c in range(N_CORES)])
    y = np.concatenate([res.results[c]["y_out"] for c in range(N_CORES)])
    s = np.concatenate([res.results[c]["s_out"] for c in range(N_CORES)])
    return (
        x.reshape(4, 32, TOPN).astype(np.int32),
        y.reshape(4, 32, TOPN).astype(np.int32),
        s.reshape(4, 32, TOPN).astype(np.float32),
    )


# revision 35
# speedup vs baseline: 1.0379x; 1.0379x over previous
"""Trainium2 Bass kernel for nn_Patchifier (grid-cell argmax + clamped top-k).

Per (b, n) map of shape [480, 640] (128 maps total, 16 per core):
  - 10x10 grid of 48x64 cells; per-cell argmax (first occurrence, row-major)
  - clamp coords to [1, 478] x [1, 638]; re-read score at the clamped coord
  - top-80 of the 100 candidate scores, sorted descending, jax tie order
Returns (x_coords i32 [4,32,80], y_coords i32, top_scores f32).

v2 dataflow per core (vs the relay/match_replace baseline):
  1. per map: HBM -> SBUF raw [120, 2560]; DVE rowmax over 64-col chunks;
     PE transpose; DRAM relay scatter (addr = 480g + 48gc + 12r + s)
  2. per map-group, batched via stride-0 broadcast APs:
     relay read-back [100 cells, k*48]; cellmax + first-attaining-row
     (value-iota min trick) -> rmin; gather chunk index = rmin*10 + const
     (one stt, no clamp needed for the argmax row); ONE indirect gather
     per map of the argmax row's 64 columns
  3. batched column argmax (same min trick), fused Relu clamp chains
     (3 Act ops per axis), then a SECOND pointwise indirect gather reads
     score[xsl, ysl] directly at the clamped coordinates -- no border-cell
     special cases anywhere
  4. top-k as a rank sort: PE broadcasts each map's 100 candidate scores
     into a [100, 100] comparison matrix (+ eps*[c'<c] tie-break constant
     accumulated on the PE; eps=4e-6 sits between the f32 ulp and the
     minimum nonzero score gap, reproducing jax's lower-index-first tie
     order), DVE counts strictly-greater entries -> rank, one-hot(rank)
     matmul gathers (s, x, y) sorted in one shot.
  5. group sizes [4,4,4,2,1,1]: big batches while streaming overlaps
     everything, tiny groups at the end to shorten the serial tail.
"""

import sys

if "/opt/trn_rl_repo" not in sys.path:
    sys.path.insert(0, "/opt/trn_rl_repo")

import numpy as np

import concourse.bacc as bacc
import concourse.bass as bass
import concourse.mybir as mybir
from concourse.bass import IndirectOffsetOnAxis, broadcast_tensor_aps
from concourse.tile import TileContext
from concourse.bass_utils import run_bass_kernel_spmd

F32 = mybir.dt.float32
I32 = mybir.dt.int32

N_CORES = 8
MAPS = 16
H, W = 480, 640
G10 = 10
CH, CW = 48, 64
NCAND = 100
TOPN = 80
GROUPS = [(0, 4), (4, 4), (8, 2), (10, 2), (12, 1), (13, 1), (14, 1), (15, 1)]
EPS_TIE = 4e-6
AX = mybir.AluOpType
AF = mybir.ActivationFunctionType

# cpack column blocks: name -> (col0, width)
_CPACK = {
    "viota48": (0, CH),       # 4*(f%12) + f//12 - 1000
    "iota64m": (48, CW),      # c - 1000
    "iota80": (112, TOPN),    # k
    "bxrow": (192, 1),        # 999 + 48g
    "bycol": (193, 1),        # 999 + 64gc
    "b477": (194, 1),
    "b637": (195, 1),
    "negcgc64": (196, 1),     # -64gc
    "ones100": (197, 1),
    "selc": (198, 1),         # rmin value that means "row got clamped"
    "iota64c": (199, CW),     # c
    "bigmoff": (263, MAPS),   # 10000 + 480g + 4800m + gc
    "iota80r": (263 + MAPS, TOPN),  # 99 - k (rank = 99 - wins)
}
_CPACK_W = 263 + MAPS + TOPN
NX = 120


def _consts():
    p = np.arange(NCAND)
    g = p // G10
    gc = p % G10
    f = np.arange(CH)
    cp = np.zeros((128, _CPACK_W), dtype=np.float32)
    cp[:, 0:48] = (4 * (f % 12) + f // 12 - 1000)[None, :]
    cp[:, 48:112] = (np.arange(CW) - 1000)[None, :]
    cp[:, 112:192] = np.arange(TOPN, dtype=np.float32)[None, :]
    cp[:NCAND, 192] = 999 + CH * g
    cp[:NCAND, 193] = 999 + CW * gc
    cp[:, 194] = 477.0
    cp[:, 195] = 637.0
    cp[:NCAND, 196] = -CW * gc
    cp[:, 197] = 1.0
    # sel: row-clamped iff rmin == -1000 (g=0) or rmin == -953 (g=9)
    cp[:NCAND, 198] = np.where(g == 0, -1000.0,
                               np.where(g == 9, -953.0, 5.0))
    cp[:, 199:263] = np.arange(CW, dtype=np.float32)[None, :]
    cp[:NCAND, 263:263 + MAPS] = (
        10000 + 480 * g[:, None] + 4800 * np.arange(MAPS)[None, :]
        + gc[:, None])
    cp[:, 263 + MAPS:263 + MAPS + TOPN] = (
        99 - np.arange(TOPN, dtype=np.float32))[None, :]
    c = {"cpack": cp}
    c["ident"] = np.eye(128, dtype=np.float32)
    c["ltepsT"] = (EPS_TIE * (p[:, None] < p[None, :])).astype(np.float32)
    e = np.zeros((4, 4 * NCAND), dtype=np.float32)
    for k in range(4):
        e[k, k * NCAND:(k + 1) * NCAND] = 1.0
    c["esel"] = e
    # pm2: [0:100, 0:120] = Pdup (dup border cells' value onto rows 100-119,
    # identity on 0-99); [0:120, 120:220] = Psel2 (select the border-row
    # score for border cells, zero for interior)
    pm2 = np.zeros((128, 220), dtype=np.float32)
    pm2[np.arange(NCAND), np.arange(NCAND)] = 1.0
    for i in range(10):
        pm2[i, 100 + i] = 1.0            # cell i -> partition 100+i
        pm2[90 + i, 110 + i] = 1.0       # cell 90+i -> partition 110+i
        pm2[100 + i, 120 + i] = 1.0      # Psel2: border row for cells 0-9
        pm2[110 + i, 120 + 90 + i] = 1.0  # ... and cells 90-99
    c["pm2"] = pm2
    # constant gather rows for partitions 100-119: row 1 (p=100+i, col
    # block gc=i) and row 478 (p=110+i); chunk idx = row*10 + gc + 4800m
    ib = np.zeros((20, MAPS), dtype=np.int32)
    for i in range(10):
        ib[i, :] = 10 + i + 4800 * np.arange(MAPS)
        ib[10 + i, :] = 4780 + i + 4800 * np.arange(MAPS)
    c["idxbrd"] = ib
    return c


_CONST_SHAPES = {
    "cpack": [128, _CPACK_W], "ident": [128, 128],
    "ltepsT": [NCAND, NCAND], "esel": [4, 4 * NCAND], "pm2": [128, 220],
}


def _bc(a, b):
    return broadcast_tensor_aps(a, b)


def build_nc():
    nc = bacc.Bacc()

    score = nc.dram_tensor("score", [MAPS, H, W], F32, kind="ExternalInput")
    cdecl = {
        name: nc.dram_tensor(name, shape, F32, kind="ExternalInput")
        for name, shape in _CONST_SHAPES.items()
    }
    x_out = nc.dram_tensor("x_out", [MAPS, TOPN], I32, kind="ExternalOutput")
    y_out = nc.dram_tensor("y_out", [MAPS, TOPN], I32, kind="ExternalOutput")
    s_out = nc.dram_tensor("s_out", [MAPS, TOPN], F32, kind="ExternalOutput")

    # chunk view (64 contiguous floats) and flat element view of the input
    score_chunks = score[:, :, :].rearrange("m r (gc c) -> (m r gc) c", c=CW)
    idxbrd = nc.dram_tensor("idxbrd", [20, MAPS], I32, kind="ExternalInput")

    # DRAM relay: addr = 480g + 48gc + 12r + s -> [cell, f=12r+s] read-back
    relay = nc.dram_tensor("relay", [MAPS, NCAND * CH], F32)

    with TileContext(nc) as tc:
        with (
            tc.tile_pool(name="raw", bufs=10) as rawp,
            tc.tile_pool(name="rm", bufs=4) as rmp,
            tc.tile_pool(name="jk", bufs=3) as jkp,
            tc.tile_pool(name="small", bufs=1) as sp,
            tc.tile_pool(name="psA", bufs=2, space="PSUM") as ppA,
            tc.tile_pool(name="psB", bufs=1, space="PSUM") as ppB,
        ):
            raw0 = rawp.tile([120, 2560], F32, tag="raw")
            nc.sync.dma_start(
                out=raw0[:, :],
                in_=score[0:1, :, :].rearrange(
                    "mm (p rr) c -> p (mm rr c)", rr=4),
            )

            ctile = {}
            for name, d in cdecl.items():
                t = sp.tile(list(d.shape), F32, tag=name)
                nc.scalar.dma_start(out=t[:, :], in_=d[:, :])
                ctile[name] = t
            ident = ctile["ident"]

            def C(name, f=None, pn=NCAND):
                c0, w = _CPACK[name]
                f0, f1 = (0, w) if f is None else f
                return ctile["cpack"][0:pn, c0 + f0:c0 + f1]

            # long-lived state
            rtall = sp.tile([40, MAPS * 120], F32, tag="rtall")
            cellsX = sp.tile([NCAND, MAPS * CH], F32, tag="cellsX")
            chunkCat = sp.tile([NX, MAPS * CW], F32, tag="chunkCat")
            cmx = sp.tile([NCAND, MAPS], F32, tag="cmx")
            rminf = sp.tile([NCAND, MAPS], F32, tag="rminf")
            xt1 = sp.tile([NCAND, MAPS], F32, tag="xt1")
            xt2 = sp.tile([NCAND, MAPS], F32, tag="xt2")
            xslX = sp.tile([NCAND, MAPS], F32, tag="xslX")
            idxf = sp.tile([NCAND, MAPS], F32, tag="idxf")
            idxi = sp.tile([NX, MAPS], I32, tag="idxi")
            nc.scalar.dma_start(out=idxi[NCAND:NX, :], in_=idxbrd[:, :])
            cm64 = sp.tile([NCAND, MAPS], F32, tag="cm64")
            cminf = sp.tile([NCAND, MAPS], F32, tag="cminf")
            yt1 = sp.tile([NCAND, MAPS], F32, tag="yt1")
            yt2 = sp.tile([NCAND, MAPS], F32, tag="yt2")
            yslY = sp.tile([NCAND, MAPS], F32, tag="yslY")
            cclv = sp.tile([NCAND, MAPS], F32, tag="cclv")
            cclx = sp.tile([NX, MAPS], F32, tag="cclx")
            scoreX = sp.tile([NX, MAPS], F32, tag="scoreX")
            selv = sp.tile([NCAND, MAPS], F32, tag="selv")
            bd = sp.tile([NCAND, MAPS], F32, tag="bd")
            be = sp.tile([NCAND, MAPS], F32, tag="be")
            S3v = sp.tile([NCAND, 3 * MAPS], F32, tag="S3v")
            sc16 = sp.tile([NCAND, MAPS], F32, tag="sc16")
            rows4s = sp.tile([4, NCAND], F32, tag="rows4s")
            rankf = sp.tile([NCAND, MAPS], F32, tag="rankf")
            srt = sp.tile([TOPN, 3 * MAPS], F32, tag="srt")

            psmall = ppB.tile([NX, 212], F32, tag="psmall")
            cclP = psmall[:, 0:MAPS]
            rows4P = psmall[0:4, 32:132]
            aallP = None  # per-group, double-banked
            sortP = ppB.tile([TOPN, 3 * MAPS], F32, tag="sortP")

            c3 = lambda t, f: t.rearrange("p (m f) -> p m f", f=f)
            c1 = lambda t: t.rearrange("p (m o) -> p m o", o=1)

            def load_map(m):
                if m == 0:
                    return raw0
                r = rawp.tile([120, 2560], F32, tag="raw")
                nc.sync.dma_start(
                    out=r[:, :],
                    in_=score[m:m + 1, :, :].rearrange(
                        "mm (p rr) c -> p (mm rr c)", rr=4),
                )
                return r

            def rowmax_stage(m, raw):
                hp = tc.high_priority()
                hp.__enter__()
                rm = rmp.tile([120, G10, 4], F32, tag="rm")
                nc.vector.reduce_max(
                    out=rm[:, :, :].rearrange("p gc r -> p r gc"),
                    in_=raw[:, :].rearrange("p (r gc c) -> p r gc c",
                                            r=4, gc=G10),
                    axis=mybir.AxisListType.X,
                )
                ps = ppA.tile([40, 120], F32, tag="ps_rt")
                nc.tensor.transpose(
                    out=ps[:, :],
                    in_=rm[:, :, :].rearrange("p gc r -> p (gc r)"),
                    identity=ident[:120, :120])
                nc.scalar.copy(
                    out=rtall[:, m * 120:(m + 1) * 120], in_=ps[:, :])
                hp.__exit__(None, None, None)

            def relay_out(m):
                nc.sync.dma_start(
                    out=relay[m:m + 1, :].rearrange(
                        "mm (g2 q s) -> (mm q) g2 s", g2=G10, q=40, s=12),
                    in_=rtall[:, m * 120:(m + 1) * 120].rearrange(
                        "q (g2 s) -> q g2 s", g2=G10),
                )

            def relay_in(m0, k):
                sl48 = slice(m0 * CH, (m0 + k) * CH)
                nc.sync.dma_start(
                    out=cellsX[:, sl48].rearrange("p (m f) -> p m f", f=CH),
                    in_=relay[m0:m0 + k, :].rearrange(
                        "mm (cell f) -> cell mm f", cell=NCAND),
                )

            def group_pre(m0, k):
                """relay-in -> rmin -> gather the argmax row's chunk."""
                sl = slice(m0, m0 + k)
                sl48 = slice(m0 * CH, (m0 + k) * CH)

                nc.vector.reduce_max(
                    out=cmx[:, sl], in_=c3(cellsX[:, sl48], CH),
                    axis=mybir.AxisListType.X)
                jR1 = jkp.tile([NCAND, 4 * CH], F32, tag="jR1")
                a, b = _bc(c3(cellsX[:, sl48], CH), c1(cmx[:, sl]))
                nc.vector.tensor_tensor(
                    out=c3(jR1[:, 0:k * CH], CH), in0=a, in1=b, op=AX.is_ge)
                jR2 = jkp.tile([NCAND, 4 * CH], F32, tag="jR2")
                a, b = _bc(c3(jR1[:, 0:k * CH], CH),
                           C("viota48").rearrange("p (o f) -> p o f", o=1))
                nc.vector.tensor_tensor(
                    out=c3(jR2[:, 0:k * CH], CH), in0=a, in1=b, op=AX.mult)
                nc.vector.tensor_reduce(
                    out=rminf[:, sl], in_=c3(jR2[:, 0:k * CH], CH),
                    axis=mybir.AxisListType.X, op=AX.min)

                # chunk idx = rmin*10 + (10000 + 480g + 4800m + gc);
                # partitions 100-119 use constant rows 1 / 478 (preloaded).
                # Per-map Act ops (scale/bias fusion) keep this off the DVE.
                for m in range(m0, m0 + k):
                    nc.scalar.activation(
                        idxf[:, m:m + 1], rminf[:, m:m + 1], AF.Relu,
                        bias=C("bigmoff", f=(m, m + 1)), scale=10.0)
                nc.vector.tensor_copy(idxi[0:NCAND, sl], idxf[:, sl])
                for m in range(m0, m0 + k):
                    nc.gpsimd.indirect_dma_start(
                        out=chunkCat[:, m * CW:(m + 1) * CW],
                        out_offset=None, in_=score_chunks,
                        in_offset=IndirectOffsetOnAxis(
                            ap=idxi[:, m:m + 1], axis=0),
                    )
            def group_post(m0, k):
                """column argmax -> clamp -> score blend -> rank."""
                ctx = tc.high_priority(offset=150)
                ctx.__enter__()
                sl = slice(m0, m0 + k)
                sl64 = slice(m0 * CW, (m0 + k) * CW)
                sl3 = slice(3 * m0, 3 * (m0 + k))

                # row clamp chain (x output; off the gather path)
                nc.scalar.activation(xt1[:, sl], rminf[:, sl], AF.Relu,
                                     bias=C("bxrow"))
                nc.scalar.activation(xt2[:, sl], xt1[:, sl], AF.Relu,
                                     bias=C("b477"), scale=-1.0)
                s3 = S3v[:, 3 * m0:3 * (m0 + k)].rearrange(
                    "p (m e) -> p m e", e=3)
                nc.scalar.activation(s3[:, :, 1:2], c1(xt2[:, sl]), AF.Copy,
                                     bias=float(H - 2), scale=-1.0)

                nc.vector.reduce_max(
                    out=cm64[:, sl], in_=c3(chunkCat[0:NCAND, sl64], CW),
                    axis=mybir.AxisListType.X)
                j641 = jkp.tile([NCAND, 4 * CW], F32, tag="j641")
                a, b = _bc(c3(chunkCat[0:NCAND, sl64], CW), c1(cm64[:, sl]))
                nc.vector.tensor_tensor(
                    out=c3(j641[:, 0:k * CW], CW), in0=a, in1=b, op=AX.is_ge)
                j642 = jkp.tile([NCAND, 4 * CW], F32, tag="j642")
                a, b = _bc(c3(j641[:, 0:k * CW], CW),
                           C("iota64m").rearrange("p (o f) -> p o f", o=1))
                nc.vector.tensor_tensor(
                    out=c3(j642[:, 0:k * CW], CW), in0=a, in1=b, op=AX.mult)
                nc.vector.tensor_reduce(
                    out=cminf[:, sl], in_=c3(j642[:, 0:k * CW], CW),
                    axis=mybir.AxisListType.X, op=AX.min)

                # col clamp chain: ysl = clip(cmin+1000+64gc, 1, 638);
                # ccl = ysl - 64gc (column within the 64-chunk)
                nc.scalar.activation(yt1[:, sl], cminf[:, sl], AF.Relu,
                                     bias=C("bycol"))
                nc.scalar.activation(yt2[:, sl], yt1[:, sl], AF.Relu,
                                     bias=C("b637"), scale=-1.0)
                s3y = S3v[:, 3 * m0:3 * (m0 + k)].rearrange(
                    "p (m e) -> p m e", e=3)[:, :, 2:3]
                nc.scalar.activation(s3y, c1(yt2[:, sl]), AF.Copy,
                                     bias=float(W - 2), scale=-1.0)
                nc.scalar.activation(c1(cclv[:, sl]), s3y, AF.Relu,
                                     bias=C("negcgc64"))

                # duplicate border cells' ccl onto partitions 100-119
                nc.tensor.matmul(
                    out=cclP[:, sl], lhsT=ctile["pm2"][0:NCAND, 0:NX],
                    rhs=cclv[:, sl], start=True, stop=True)
                nc.scalar.copy(out=cclx[:, sl], in_=cclP[:, sl])

                # score extraction on all 120 rows: one-hot(ccl) . chunk
                jS1 = jkp.tile([NX, 4 * CW], F32, tag="jS1")
                a, b = _bc(C("iota64c", pn=NX).rearrange(
                    "p (o f) -> p o f", o=1), c1(cclx[:, sl]))
                nc.vector.tensor_tensor(
                    out=c3(jS1[:, 0:k * CW], CW), in0=a, in1=b,
                    op=AX.is_equal)
                jS2 = jkp.tile([NX, 4 * CW], F32, tag="jS2")
                nc.vector.tensor_tensor(
                    out=jS2[:, 0:k * CW], in0=jS1[:, 0:k * CW],
                    in1=chunkCat[:, sl64], op=AX.mult)
                nc.vector.tensor_reduce(
                    out=scoreX[:, sl], in_=c3(jS2[:, 0:k * CW], CW),
                    axis=mybir.AxisListType.X, op=AX.add)
                # border-row score per cell (zero for interior)
                sbP = psmall[:, 16:32]
                nc.tensor.matmul(
                    out=sbP[0:NCAND, sl], lhsT=ctile["pm2"][0:NX, 120:220],
                    rhs=scoreX[:, sl], start=True, stop=True)
                # blend: row-clamped cells take the border-row value
                a, b = _bc(c1(rminf[:, sl]), C("selc").rearrange(
                    "p (o f) -> p o f", o=1))
                nc.vector.tensor_tensor(
                    out=c1(selv[:, sl]), in0=a, in1=b, op=AX.is_equal)
                nc.vector.tensor_tensor(
                    out=bd[:, sl], in0=sbP[0:NCAND, sl],
                    in1=scoreX[0:NCAND, sl], op=AX.subtract)
                nc.vector.tensor_tensor(
                    out=be[:, sl], in0=bd[:, sl], in1=selv[:, sl],
                    op=AX.mult)
                nc.vector.tensor_tensor(
                    out=sc16[:, sl], in0=be[:, sl],
                    in1=scoreX[0:NCAND, sl], op=AX.add)
                s3s = S3v[:, 3 * m0:3 * (m0 + k)].rearrange(
                    "p (m e) -> p m e", e=3)[:, :, 0:1]
                nc.vector.tensor_copy(s3s, c1(sc16[:, sl]))

                ctx.__exit__(None, None, None)

            def rank_stage(m0, k):
                sl = slice(m0, m0 + k)
                nc.tensor.matmul(
                    out=rows4P[0:k, :], lhsT=sc16[:, sl],
                    rhs=ident[:NCAND, :NCAND], start=True, stop=True)
                nc.scalar.copy(out=rows4s[0:k, :], in_=rows4P[0:k, :])
                aallP = ppB.tile([NCAND, 4 * NCAND], F32, tag="aallP",
                                 bufs=2)
                for j in range(k):
                    asl = slice(j * NCAND, (j + 1) * NCAND)
                    nc.tensor.matmul(
                        out=aallP[:, asl],
                        lhsT=ctile["esel"][0:k, asl],
                        rhs=rows4s[0:k, :], start=True, stop=False)
                    nc.tensor.matmul(
                        out=aallP[:, asl], lhsT=ctile["ltepsT"][:, :],
                        rhs=ident[:NCAND, :NCAND], start=False, stop=True)
                jG = jkp.tile([NCAND, 4 * NCAND], F32, tag="jG")
                a, b = _bc(c3(aallP[:, 0:k * NCAND], NCAND), c1(sc16[:, sl]))
                nc.vector.tensor_tensor(
                    out=c3(jG[:, 0:k * NCAND], NCAND), in0=a, in1=b,
                    op=AX.is_gt)
                nc.vector.tensor_reduce(
                    out=rankf[:, sl], in_=c3(jG[:, 0:k * NCAND], NCAND),
                    axis=mybir.AxisListType.X, op=AX.add)
                jO = jkp.tile([NCAND, 4 * TOPN], F32, tag="jO")
                a, b = _bc(C("iota80").rearrange("p (o f) -> p o f", o=1),
                           c1(rankf[:, sl]))
                nc.vector.tensor_tensor(
                    out=c3(jO[:, 0:k * TOPN], TOPN), in0=a, in1=b,
                    op=AX.is_equal)
                for j in range(k):
                    m = m0 + j
                    nc.tensor.matmul(
                        out=sortP[:, 3 * m:3 * m + 3],
                        lhsT=jO[:, j * TOPN:(j + 1) * TOPN],
                        rhs=S3v[:, 3 * m:3 * m + 3],
                        start=True, stop=True)

            # ---- main schedule ----
            # All HWDGE DMAs ride the SP queue in the exact order they
            # should win DMA_ENGINES: relay-out(m) two loads after map m,
            # relay-in(g) one load later still, so no wait ever blocks the
            # load stream and small DMAs are never starved by the big loads.
            ri_at = {GROUPS[gi][0] + GROUPS[gi][1] - 1 + 3: gi
                     for gi in range(len(GROUPS))}
            emitted_post = 0

            def emit_step(m):
                nonlocal emitted_post
                if m < MAPS:
                    raw = load_map(m)
                    rowmax_stage(m, raw)
                if 0 <= m - 2 < MAPS:
                    relay_out(m - 2)
                gi = ri_at.get(m)
                if gi is not None:
                    relay_in(*GROUPS[gi])
                    if gi >= 1:
                        group_post(*GROUPS[gi - 1])
                        emitted_post = gi
                    group_pre(*GROUPS[gi])

            for m in range(MAPS + 3 + len(GROUPS)):
                emit_step(m)
                if emitted_post == len(GROUPS) - 1:
                    break
            group_post(*GROUPS[-1])
            for r0, rk in [(0, 4), (4, 4), (8, 4), (12, 1), (13, 1),
                           (14, 1), (15, 1)]:
                rank_stage(r0, rk)

            # ---- tail: PE-transpose the sorted tile, then 3 contiguous
            # DMAs (partition stride 3 picks out s / x / y rows) ----
            nc.scalar.copy(out=srt[:, :], in_=sortP[:, :])
            outP = psmall[0:3 * MAPS, 132:212]
            nc.tensor.transpose(
                out=outP[:, :], in_=srt[:, :], identity=ident[:TOPN, :TOPN])
            outS = sp.tile([3 * MAPS, TOPN], F32, tag="outS")
            nc.scalar.copy(out=outS[:, :], in_=outP[:, :])
            outI = sp.tile([3 * MAPS, TOPN], I32, tag="outI")
            nc.vector.tensor_copy(outI[:, :], outS[:, :])
            o3f = outS[:, :].rearrange("(m e) k -> m e k", e=3)
            o3i = outI[:, :].rearrange("(m e) k -> m e k", e=3)
            nc.sync.dma_start(
                out=s_out[:, :].rearrange("m (o k) -> m o k", o=1),
                in_=o3f[:, 0:1, :])
            nc.scalar.dma_start(
                out=x_out[:, :].rearrange("m (o k) -> m o k", o=1),
                in_=o3i[:, 1:2, :])
            nc.gpsimd.dma_start(
                out=y_out[:, :].rearrange("m (o k) -> m o k", o=1),
                in_=o3i[:, 2:3, :])

    nc.compile()
    return nc


_NC = None


def _get_nc():
    global _NC
    if _NC is None:
        _NC = build_nc()
    return _NC


def kernel(score_maps: np.ndarray, top_n=80, _trace=False):
    score_maps = np.ascontiguousarray(np.asarray(score_maps), dtype=np.float32)
    assert score_maps.shape == (4, 32, H, W), score_maps.shape
    assert int(top_n) == TOPN

    nc = _get_nc()
    consts = _consts()
    flat = score_maps.reshape(4 * 32, H, W)
    in_maps = []
    for c in range(N_CORES):
        m = {"score": np.ascontiguousarray(flat[c * MAPS:(c + 1) * MAPS])}
        m.update(consts)
        in_maps.append(m)

    res = run_bass_kernel_spmd(nc, in_maps, list(range(N_CORES)), trace=_trace)
    x = np.concatenate([res.results[c]["x_out"] for c in range(N_CORES)])
    y = np.concatenate([res.results[c]["y_out"] for c in range(N_CORES)])
    s = np.concatenate([res.results[c]["s_out"] for c in range(N_CORES)])
    return (
        x.reshape(4, 32, TOPN).astype(np.int32),
        y.reshape(4, 32, TOPN).astype(np.int32),
        s.reshape(4, 32, TOPN).astype(np.float32),
    )

